# revision 4
# baseline (speedup 1.0000x reference)
"""CrossAttention kernel for 8 Trainium2 NeuronCores.

Sharding: core c -> batch b = c//2, head-half hh = c%2 (8 of 16 heads).
Each core computes q/k/v projections for its (batch, head-half), the
attention for its 8 heads, and a partial output projection. Host sums the
two partial outputs per batch and adds the bias.

Layout strategy: all on-chip activations keep the contraction dim on
partitions. Host feeds x/y pre-transposed (xT[c, n] = x[n, c]) so no
on-chip transposes are needed. Attention scores are computed transposed
(sT[n2, n1]) so the ctx matmul needs no attn transpose; softmax row sums
come for free from a ones-column appended to v. attn is returned
per-head-transposed and host transposes it back.

Matmuls run in float32r (TF32-like, ~1e-4 rel err, 4x faster than fp32).
"""
import numpy as np

_CACHE = {}

P = 128
N = 1024          # sequence length (N1 == N2)
C = 1024          # model dim
HL = 8            # heads per core
HD = 64           # head dim
COUT = HL * HD    # 512, per-core projection width
SCALE = HD ** -0.5


def _build():
    import concourse.bacc as bacc
    import concourse.mybir as mybir
    import concourse.tile as tile

    f32 = mybir.dt.float32
    f32r = mybir.dt.float32r
    AF = mybir.ActivationFunctionType

    nc = bacc.Bacc("TRN2", target_bir_lowering=False, debug=False, num_devices=8)

    xT = nc.dram_tensor("xT", [8, P, N], f32r, kind="ExternalInput").ap()
    yT = nc.dram_tensor("yT", [8, P, N], f32r, kind="ExternalInput").ap()
    wq = nc.dram_tensor("wq", [8, P, COUT], f32r, kind="ExternalInput").ap()
    wk = nc.dram_tensor("wk", [8, P, COUT], f32r, kind="ExternalInput").ap()
    wv = nc.dram_tensor("wv", [8, P, COUT], f32r, kind="ExternalInput").ap()
    wproj = nc.dram_tensor("wproj", [4, P, C], f32r, kind="ExternalInput").ap()
    attnT = nc.dram_tensor("attnT", [HL, N, N], f32, kind="ExternalOutput").ap()
    outp = nc.dram_tensor("outp", [N, C], f32, kind="ExternalOutput").ap()

    with tile.TileContext(nc) as tc:
        with tc.tile_pool(name="persist", bufs=1) as pp, \
             tc.tile_pool(name="exp", bufs=2) as ep, \
             tc.tile_pool(name="rb", bufs=1) as rbp, \
             tc.tile_pool(name="small", bufs=1) as smp, \
             tc.tile_pool(name="consts", bufs=1) as cop, \
             tc.tile_pool(name="outb", bufs=2) as obp, \
             tc.tile_pool(name="psmm", bufs=3, space="PSUM") as ps_mm, \
             tc.tile_pool(name="pss", bufs=3, space="PSUM") as ps_s, \
             tc.tile_pool(name="psctx", bufs=2, space="PSUM") as ps_ctx:

            # ---- inputs ----
            yT_sb = pp.tile([P, 8, N], f32r, tag="yT")
            nc.sync.dma_start(yT_sb[:], yT.rearrange("k p n -> p k n"))
            xT_sb = pp.tile([P, 8, N], f32r, tag="xT")
            nc.sync.dma_start(xT_sb[:], xT.rearrange("k p n -> p k n"))
            wk_sb = pp.tile([P, 8, COUT], f32r, tag="wkp")  # slot reused by wproj
            nc.sync.dma_start(wk_sb[:], wk.rearrange("k p m -> p k m"))
            wv_sb = ep.tile([P, 8, COUT], f32r, tag="exp")  # shares exp slots
            nc.sync.dma_start(wv_sb[:], wv.rearrange("k p m -> p k m"))
            wq_sb = pp.tile([P, 8, COUT], f32r, tag="wq")
            nc.sync.dma_start(wq_sb[:], wq.rearrange("k p m -> p k m"))

            # ---- constants (scratch borrows the rb slot) ----
            scratch = rbp.tile([P, 512], f32, tag="rb")
            nc.gpsimd.memset(scratch[:, 0:P], 1.0)
            ones64 = cop.tile([P, 64], f32r, tag="ones64")
            nc.vector.tensor_copy(ones64[:], scratch[:, 0:64])
            onerow = cop.tile([1, P], f32r, tag="onerow")
            nc.vector.tensor_copy(onerow[:], scratch[0:1, 0:P])

            # ---- projections ----
            kT_sb = pp.tile([P, 4, N], f32r, tag="kT")      # [cout, n2], m-tile major
            qT_sb = pp.tile([P, 4, N], f32r, tag="qT")      # [cout, n1]
            vones = pp.tile([P, 8, HL * 65], f32r, tag="vones")  # [n2, v|1 per head]
            nc.vector.tensor_copy(
                vones[:].rearrange("p k (h e) -> p (k h) e", e=65)[:, :, 64],
                ones64[:])

            # kT = Wk^T @ y^T
            for mt in range(4):
                for nh in range(2):
                    mm_ps = ps_mm.tile([P, 512], f32, tag="mm")
                    for k in range(8):
                        nc.tensor.matmul(
                            mm_ps[:],
                            wk_sb[:, k, mt * P:(mt + 1) * P],
                            yT_sb[:, k, nh * 512:(nh + 1) * 512],
                            start=(k == 0), stop=(k == 7))
                    nc.vector.tensor_copy(
                        kT_sb[:, mt, nh * 512:(nh + 1) * 512], mm_ps[:])
            # v natural: [n2, d] per head, interleaved with ones columns
            for t in range(8):
                mm_ps = ps_mm.tile([P, 512], f32, tag="mm")
                for k in range(8):
                    nc.tensor.matmul(
                        mm_ps[:],
                        yT_sb[:, k, t * P:(t + 1) * P],
                        wv_sb[:, k, :],
                        start=(k == 0), stop=(k == 7))
                nc.vector.tensor_copy(
                    vones[:, t, :].rearrange("p (h e) -> p h e", e=65)[:, :, 0:64],
                    mm_ps[:].rearrange("p (h d) -> p h d", d=64))
            # qT = Wq^T @ x^T
            for mt in range(4):
                for nh in range(2):
                    mm_ps = ps_mm.tile([P, 512], f32, tag="mm")
                    for k in range(8):
                        nc.tensor.matmul(
                            mm_ps[:],
                            wq_sb[:, k, mt * P:(mt + 1) * P],
                            xT_sb[:, k, nh * 512:(nh + 1) * 512],
                            start=(k == 0), stop=(k == 7))
                    nc.vector.tensor_copy(
                        qT_sb[:, mt, nh * 512:(nh + 1) * 512], mm_ps[:])

            # prefetch wproj into the freed wk slot (overlaps attention)
            wproj_sb = pp.tile([P, 4, C], f32r, tag="wkp")
            nc.sync.dma_start(wproj_sb[:], wproj.rearrange("k p m -> p k m"))

            ctxT = pp.tile([P, 4, N], f32r, tag="ctxT")     # [d, n1], chunk major

            # ---- attention, per (head, n1-half) ----
            for h in range(HL):
                th, po = h // 2, (h % 2) * 64
                for nh in range(2):
                    n1s = nh * 512
                    exp_t = ep.tile([P, 8, 512], f32r, tag="exp")
                    for n2c in range(8):
                        s_ps = ps_s.tile([P, 512], f32, tag="s")
                        nc.tensor.matmul(
                            s_ps[:],
                            kT_sb[po:po + 64, th, n2c * P:(n2c + 1) * P],
                            qT_sb[po:po + 64, th, n1s:n1s + 512],
                            start=True, stop=True)
                        nc.scalar.activation(
                            exp_t[:, n2c, :], s_ps[:], AF.Exp, scale=SCALE)
                    ctx_ps = ps_ctx.tile([65, 512], f32, tag="ctx")
                    for n2c in range(8):
                        nc.tensor.matmul(
                            ctx_ps[:],
                            vones[:, n2c, h * 65:(h + 1) * 65],
                            exp_t[:, n2c, :],
                            start=(n2c == 0), stop=(n2c == 7))
                    r_sb = smp.tile([1, 512], f32r, tag="r")
                    with nc.allow_low_precision(reason="f32r feeds PE broadcast"):
                        nc.vector.reciprocal(r_sb[:], ctx_ps[64:65, :])
                    rb_ps = ps_mm.tile([P, 512], f32, tag="mm")
                    nc.tensor.matmul(rb_ps[:], onerow[:], r_sb[:],
                                     start=True, stop=True)
                    rb_sb = rbp.tile([P, 512], f32, tag="rb")
                    nc.vector.tensor_copy(rb_sb[:], rb_ps[:])
                    nc.vector.tensor_mul(
                        ctxT[po:po + 64, th, n1s:n1s + 512],
                        ctx_ps[0:64, :], rb_sb[0:64, :])
                    for n2c in range(8):
                        nc.vector.tensor_mul(
                            exp_t[:, n2c, :], exp_t[:, n2c, :], rb_sb[:])
                        nc.sync.dma_start(
                            attnT[h, n2c * P:(n2c + 1) * P, n1s:n1s + 512],
                            exp_t[:, n2c, :].bitcast(f32))

            # ---- output projection (partial; host adds halves + bias) ----
            for t in range(8):
                o_sb = obp.tile([P, C], f32, tag="o")
                for co in range(2):
                    mm_ps = ps_mm.tile([P, 512], f32, tag="mm")
                    for ch in range(4):
                        nc.tensor.matmul(
                            mm_ps[:],
                            ctxT[:, ch, t * P:(t + 1) * P],
                            wproj_sb[:, ch, co * 512:(co + 1) * 512],
                            start=(ch == 0), stop=(ch == 3))
                    nc.vector.tensor_copy(o_sb[:, co * 512:(co + 1) * 512], mm_ps[:])
                nc.sync.dma_start(outp[t * P:(t + 1) * P, :], o_sb[:])

    nc.compile()
    return nc


def _in_maps(x, y, Wq, Wkv, Wproj):
    x = np.asarray(x, np.float32)
    y = np.asarray(y, np.float32)
    Wq = np.asarray(Wq, np.float32)
    Wkv = np.asarray(Wkv, np.float32)
    Wproj = np.asarray(Wproj, np.float32)
    maps = []
    for c in range(8):
        b, hh = c // 2, c % 2
        cs = slice(hh * COUT, (hh + 1) * COUT)
        maps.append({
            "xT": np.ascontiguousarray(x[b].T).reshape(8, P, N),
            "yT": np.ascontiguousarray(y[b].T).reshape(8, P, N),
            "wq": np.ascontiguousarray(Wq[:, cs]).reshape(8, P, COUT),
            "wk": np.ascontiguousarray(Wkv[:, hh * COUT:(hh + 1) * COUT]).reshape(8, P, COUT),
            "wv": np.ascontiguousarray(Wkv[:, C + hh * COUT:C + (hh + 1) * COUT]).reshape(8, P, COUT),
            "wproj": np.ascontiguousarray(Wproj[cs, :]).reshape(4, P, C),
        })
    return maps


def _assemble(results, y, bproj):
    bproj = np.asarray(bproj, np.float32)
    out = np.empty((4, N, C), np.float32)
    attn = np.empty((4, 16, N, N), np.float32)
    for c in range(8):
        b, hh = c // 2, c % 2
        attn[b, hh * HL:(hh + 1) * HL] = results[c]["attnT"].transpose(0, 2, 1)
    for b in range(4):
        out[b] = results[2 * b]["outp"] + results[2 * b + 1]["outp"] + bproj
    return out, np.asarray(y, np.float32), attn


def kernel(x, y, Wq, Wkv, Wproj, bproj):
    from concourse.bass_utils import run_bass_kernel_spmd
    if "nc" not in _CACHE:
        _CACHE["nc"] = _build()
    res = run_bass_kernel_spmd(_CACHE["nc"], _in_maps(x, y, Wq, Wkv, Wproj),
                               list(range(8)))
    return _assemble(res.results, y, bproj)


# revision 5
# speedup vs baseline: 1.1754x; 1.1754x over previous
"""CrossAttention kernel for 8 Trainium2 NeuronCores.

Sharding: core c -> batch b = c//2, head-half hh = c%2 (8 of 16 heads).
Each core computes q/k/v projections for its (batch, head-half), the
attention for its 8 heads, and a partial output projection. Host sums the
two partial outputs per batch and adds the bias.

Layout strategy: all on-chip activations keep the contraction dim on
partitions. Host feeds x/y pre-transposed (xT[c, n] = x[n, c]) so no
on-chip transposes are needed. Attention scores are computed transposed
(sT[n2, n1]) so the ctx matmul needs no attn transpose; softmax row sums
come for free from a ones-column appended to v. attn is returned
per-head-transposed and host transposes it back.

Matmuls run in float32r (TF32-like, ~1e-4 rel err, 4x faster than fp32).
"""
import numpy as np

_CACHE = {}

P = 128
N = 1024          # sequence length (N1 == N2)
C = 1024          # model dim
HL = 8            # heads per core
HD = 64           # head dim
COUT = HL * HD    # 512, per-core projection width
SCALE = HD ** -0.5


def _build():
    import concourse.bacc as bacc
    import concourse.mybir as mybir
    import concourse.tile as tile

    f32 = mybir.dt.float32
    f32r = mybir.dt.float32r
    AF = mybir.ActivationFunctionType

    nc = bacc.Bacc("TRN2", target_bir_lowering=False, debug=False, num_devices=8)

    xT = nc.dram_tensor("xT", [8, P, N], f32r, kind="ExternalInput").ap()
    yT = nc.dram_tensor("yT", [8, P, N], f32r, kind="ExternalInput").ap()
    wq = nc.dram_tensor("wq", [8, P, COUT], f32r, kind="ExternalInput").ap()
    wk = nc.dram_tensor("wk", [8, P, COUT], f32r, kind="ExternalInput").ap()
    wv = nc.dram_tensor("wv", [8, P, COUT], f32r, kind="ExternalInput").ap()
    wproj = nc.dram_tensor("wproj", [4, P, C], f32r, kind="ExternalInput").ap()
    attnT = nc.dram_tensor("attnT", [HL, N, N], f32, kind="ExternalOutput").ap()
    outp = nc.dram_tensor("outp", [N, C], f32, kind="ExternalOutput").ap()

    with tile.TileContext(nc) as tc:
        with tc.tile_pool(name="persist", bufs=1) as pp, \
             tc.tile_pool(name="exp", bufs=2) as ep, \
             tc.tile_pool(name="rb", bufs=1) as rbp, \
             tc.tile_pool(name="small", bufs=1) as smp, \
             tc.tile_pool(name="consts", bufs=1) as cop, \
             tc.tile_pool(name="outb", bufs=2) as obp, \
             tc.tile_pool(name="psmm", bufs=3, space="PSUM") as ps_mm, \
             tc.tile_pool(name="pss", bufs=3, space="PSUM") as ps_s, \
             tc.tile_pool(name="psctx", bufs=2, space="PSUM") as ps_ctx:

            # ---- inputs ----
            yT_sb = pp.tile([P, 8, N], f32r, tag="yT")
            xT_sb = pp.tile([P, 8, N], f32r, tag="xT")
            wk_sb = pp.tile([P, 8, COUT], f32r, tag="wkp")  # slot reused by wproj
            wv_sb = ep.tile([P, 8, COUT], f32r, tag="exp")  # shares exp slots
            wq_sb = pp.tile([P, 8, COUT], f32r, tag="wq")
            for k in range(8):
                nc.sync.dma_start(yT_sb[:, k, :], yT[k])
                nc.sync.dma_start(wk_sb[:, k, :], wk[k])
                nc.sync.dma_start(wv_sb[:, k, :], wv[k])
                nc.sync.dma_start(xT_sb[:, k, :], xT[k])
                nc.sync.dma_start(wq_sb[:, k, :], wq[k])

            # ---- constants (scratch borrows the rb slot) ----
            scratch = rbp.tile([P, 512], f32, tag="rb")
            nc.gpsimd.memset(scratch[:, 0:P], 1.0)
            ones64 = cop.tile([P, 64], f32r, tag="ones64")
            nc.vector.tensor_copy(ones64[:], scratch[:, 0:64])
            onerow = cop.tile([1, P], f32r, tag="onerow")
            nc.vector.tensor_copy(onerow[:], scratch[0:1, 0:P])

            # ---- projections ----
            kT_sb = pp.tile([P, 4, N], f32r, tag="kT")      # [cout, n2], m-tile major
            qT_sb = pp.tile([P, 4, N], f32r, tag="qT")      # [cout, n1]
            vones = pp.tile([P, 8, HL * 65], f32r, tag="vones")  # [n2, v|1 per head]
            nc.vector.tensor_copy(
                vones[:].rearrange("p k (h e) -> p (k h) e", e=65)[:, :, 64],
                ones64[:])

            # kT = Wk^T @ y^T
            for mt in range(4):
                for nh in range(2):
                    mm_ps = ps_mm.tile([P, 512], f32, tag="mm")
                    for k in range(8):
                        nc.tensor.matmul(
                            mm_ps[:],
                            wk_sb[:, k, mt * P:(mt + 1) * P],
                            yT_sb[:, k, nh * 512:(nh + 1) * 512],
                            start=(k == 0), stop=(k == 7))
                    nc.scalar.copy(
                        kT_sb[:, mt, nh * 512:(nh + 1) * 512], mm_ps[:])
            # v natural: [n2, d] per head, interleaved with ones columns
            for t in range(8):
                mm_ps = ps_mm.tile([P, 512], f32, tag="mm")
                for k in range(8):
                    nc.tensor.matmul(
                        mm_ps[:],
                        yT_sb[:, k, t * P:(t + 1) * P],
                        wv_sb[:, k, :],
                        start=(k == 0), stop=(k == 7))
                nc.scalar.copy(
                    vones[:, t, :].rearrange("p (h e) -> p h e", e=65)[:, :, 0:64],
                    mm_ps[:].rearrange("p (h d) -> p h d", d=64))
            # qT = Wq^T @ x^T
            for mt in range(4):
                for nh in range(2):
                    mm_ps = ps_mm.tile([P, 512], f32, tag="mm")
                    for k in range(8):
                        nc.tensor.matmul(
                            mm_ps[:],
                            wq_sb[:, k, mt * P:(mt + 1) * P],
                            xT_sb[:, k, nh * 512:(nh + 1) * 512],
                            start=(k == 0), stop=(k == 7))
                    nc.scalar.copy(
                        qT_sb[:, mt, nh * 512:(nh + 1) * 512], mm_ps[:])

            # prefetch wproj into the freed wk slot (overlaps attention)
            wproj_sb = pp.tile([P, 4, C], f32r, tag="wkp")
            nc.sync.dma_start(wproj_sb[:], wproj.rearrange("k p m -> p k m"))

            ctxT = pp.tile([P, 4, N], f32r, tag="ctxT")     # [d, n1], chunk major

            # ---- attention: n1-half outer so proj(t<4) overlaps nh=1 ----
            for nh in range(2):
                n1s = nh * 512
                for h in range(HL):
                    th, po = h // 2, (h % 2) * 64
                    exp_t = ep.tile([P, 8, 512], f32r, tag="exp")
                    for n2c in range(8):
                        s_ps = ps_s.tile([P, 512], f32, tag="s")
                        nc.tensor.matmul(
                            s_ps[:],
                            kT_sb[po:po + 64, th, n2c * P:(n2c + 1) * P],
                            qT_sb[po:po + 64, th, n1s:n1s + 512],
                            start=True, stop=True)
                        nc.scalar.activation(
                            exp_t[:, n2c, :], s_ps[:], AF.Exp, scale=SCALE)
                    ctx_ps = ps_ctx.tile([65, 512], f32, tag="ctx")
                    for n2c in range(8):
                        nc.tensor.matmul(
                            ctx_ps[:],
                            vones[:, n2c, h * 65:(h + 1) * 65],
                            exp_t[:, n2c, :],
                            start=(n2c == 0), stop=(n2c == 7))
                    r_sb = smp.tile([1, 512], f32r, tag="r")
                    with nc.allow_low_precision(reason="f32r feeds PE broadcast"):
                        nc.vector.reciprocal(r_sb[:], ctx_ps[64:65, :])
                    rb_ps = ps_mm.tile([P, 512], f32, tag="mm")
                    nc.tensor.matmul(rb_ps[:], onerow[:], r_sb[:],
                                     start=True, stop=True)
                    rb_sb = rbp.tile([64, 512], f32, tag="rb")
                    nc.scalar.copy(rb_sb[:], rb_ps[0:64, :])
                    nc.vector.tensor_mul(
                        ctxT[po:po + 64, th, n1s:n1s + 512],
                        ctx_ps[0:64, :], rb_sb[:])
                    for n2c in range(8):
                        nc.vector.tensor_mul(
                            exp_t[:, n2c, :], exp_t[:, n2c, :], rb_ps[:])
                        nc.sync.dma_start(
                            attnT[h, n2c * P:(n2c + 1) * P, n1s:n1s + 512],
                            exp_t[:, n2c, :].bitcast(f32))

                # ---- output projection for this n1-half ----
                for t in range(4 * nh, 4 * nh + 4):
                    o_sb = obp.tile([P, C], f32, tag="o")
                    for co in range(2):
                        mm_ps = ps_mm.tile([P, 512], f32, tag="mm")
                        for ch in range(4):
                            nc.tensor.matmul(
                                mm_ps[:],
                                ctxT[:, ch, t * P:(t + 1) * P],
                                wproj_sb[:, ch, co * 512:(co + 1) * 512],
                                start=(ch == 0), stop=(ch == 3))
                        nc.vector.tensor_copy(o_sb[:, co * 512:(co + 1) * 512], mm_ps[:])
                    nc.sync.dma_start(outp[t * P:(t + 1) * P, :], o_sb[:])

    nc.compile()
    return nc


def _in_maps(x, y, Wq, Wkv, Wproj):
    x = np.asarray(x, np.float32)
    y = np.asarray(y, np.float32)
    Wq = np.asarray(Wq, np.float32)
    Wkv = np.asarray(Wkv, np.float32)
    Wproj = np.asarray(Wproj, np.float32)
    maps = []
    for c in range(8):
        b, hh = c // 2, c % 2
        cs = slice(hh * COUT, (hh + 1) * COUT)
        maps.append({
            "xT": np.ascontiguousarray(x[b].T).reshape(8, P, N),
            "yT": np.ascontiguousarray(y[b].T).reshape(8, P, N),
            "wq": np.ascontiguousarray(Wq[:, cs]).reshape(8, P, COUT),
            "wk": np.ascontiguousarray(Wkv[:, hh * COUT:(hh + 1) * COUT]).reshape(8, P, COUT),
            "wv": np.ascontiguousarray(Wkv[:, C + hh * COUT:C + (hh + 1) * COUT]).reshape(8, P, COUT),
            "wproj": np.ascontiguousarray(Wproj[cs, :]).reshape(4, P, C),
        })
    return maps


def _assemble(results, y, bproj):
    bproj = np.asarray(bproj, np.float32)
    out = np.empty((4, N, C), np.float32)
    attn = np.empty((4, 16, N, N), np.float32)
    for c in range(8):
        b, hh = c // 2, c % 2
        attn[b, hh * HL:(hh + 1) * HL] = results[c]["attnT"].transpose(0, 2, 1)
    for b in range(4):
        out[b] = results[2 * b]["outp"] + results[2 * b + 1]["outp"] + bproj
    return out, np.asarray(y, np.float32), attn


def kernel(x, y, Wq, Wkv, Wproj, bproj):
    from concourse.bass_utils import run_bass_kernel_spmd
    if "nc" not in _CACHE:
        _CACHE["nc"] = _build()
    res = run_bass_kernel_spmd(_CACHE["nc"], _in_maps(x, y, Wq, Wkv, Wproj),
                               list(range(8)))
    return _assemble(res.results, y, bproj)
